# revision 1
# baseline (speedup 1.0000x reference)
"""GCN (3-layer, PyG-style) on 8 Trainium2 NeuronCores.

Strategy: dst-shard nodes across cores (degree-balanced permutation).
Per conv layer, each core gathers fp16 message rows (dma_gather) for edges
targeting its node range, segment-sums them via one-hot matmuls on the PE
(PSUM accumulation), applies deg-normalization/bias/relu, computes the next
layer's gather table shard (x' @ W), and AllGathers the table. Pooling uses
the same gather + one-hot machinery over graphs; the small MLP + softmax run
on-device; host concatenates per-core [32,2] outputs.
"""
import numpy as np

# problem constants (hardcoded per spec)
N = 50000
E = 800000
F = 128
G = 256
NCORES = 8
NPC = N // NCORES            # 6250 nodes per core
TPC = 49                     # node tiles per core: 48 x 128 + 1 x 106
LAST_CAP = NPC - 48 * 128    # 106
LO = 32768                   # int16 gather index boundary
PAD_D = 255.0                # d_local pad value (never matches iota)
MAX_IDXS = 12288             # per-gather num_idxs cap (Q7 scratch)


def _tile_caps():
    return [128] * 48 + [LAST_CAP]


def _assign_slots(deg):
    """Greedy degree-balanced assignment of nodes to (core, tile, lane) slots.
    Returns slot_of_node [N] (global permuted id)."""
    import heapq
    caps = _tile_caps()
    ntiles = NCORES * TPC
    cap_arr = np.array([caps[t % TPC] for t in range(ntiles)])
    order = np.argsort(-deg, kind="stable")
    heap = [(0, tg) for tg in range(ntiles)]
    heapq.heapify(heap)
    fill = np.zeros(ntiles, np.int64)
    slot = np.empty(N, np.int64)
    for n in order:
        popped = []
        while True:
            s, tg = heapq.heappop(heap)
            if fill[tg] < cap_arr[tg]:
                break
            popped.append((s, tg))
        c, t = tg // TPC, tg % TPC
        slot[n] = c * NPC + t * 128 + fill[tg]
        fill[tg] += 1
        if fill[tg] < cap_arr[tg]:
            heapq.heappush(heap, (s + deg[n], tg))
        for it in popped:
            pass  # fully filled tiles stay out of the heap
    assert (fill == cap_arr).all()
    return slot


def _pack_idx16(flat):
    """flat int16 [n] (n % 16 == 0) -> [128, n/16] wrapped+replicated layout."""
    a = flat.reshape(-1, 16).T.astype(np.int16)  # [16, n/16]
    return np.tile(a, (8, 1))


def preprocess(edge_index, batch):
    """All host-side index preprocessing. Returns a dict of per-core and
    shared metadata plus the static schedule constants."""
    src = np.asarray(edge_index[0], dtype=np.int64)
    dst = np.asarray(edge_index[1], dtype=np.int64)
    bat = np.asarray(batch, dtype=np.int64)

    deg = np.bincount(dst, minlength=N).astype(np.int64)
    slot = _assign_slots(deg)

    ps = slot[src]                      # permuted src id
    pd = slot[dst]
    core = pd // NPC
    loc = pd % NPC
    t = np.minimum(loc // 128, 48)
    d_local = loc - t * 128
    is_hi = ps >= LO

    # per (core, tile, half) counts -> static chunk counts
    key = (core * TPC + t) * 2 + is_hi
    cnt = np.bincount(key, minlength=NCORES * TPC * 2).reshape(NCORES, TPC, 2)
    C_LO = int(np.ceil(cnt[:, :, 0].max() / 128))
    C_HI = int(np.ceil(cnt[:, :, 1].max() / 128))
    CT = C_LO + C_HI
    ES = TPC * CT * 128   # edge slots per core

    # order edges by (core, tile, half); stable so deterministic
    eo = np.lexsort((is_hi, t, core))
    ps_o, d_o, core_o, t_o, hi_o = ps[eo], d_local[eo], core[eo], t[eo], is_hi[eo]
    # position within its (core,tile,half) group
    k_o = key[eo]
    grp_start = np.zeros(NCORES * TPC * 2 + 1, np.int64)
    np.cumsum(np.bincount(k_o, minlength=NCORES * TPC * 2), out=grp_start[1:])
    pos = np.arange(E) - grp_start[k_o]

    # flat slot position inside the per-core idx stream
    base_lo = t_o * (C_LO * 128)
    base_hi = TPC * C_LO * 128 + t_o * (C_HI * 128)
    flat = np.where(hi_o, base_hi, base_lo) + pos

    idx16 = np.zeros((NCORES, ES), np.int16)
    idxval = np.where(hi_o, ps_o - LO, ps_o).astype(np.int16)
    idx16[core_o, flat] = idxval
    dval = np.full((NCORES, 128, TPC * CT), PAD_D, np.float16)
    # chunk column for (tile, half, chunk-within-half)
    q = pos // 128
    lane = pos % 128
    col = t_o * CT + np.where(hi_o, C_LO + q, q)
    dval[core_o, lane, col] = d_o.astype(np.float16)

    # ---- pooling metadata ----
    # nodes grouped by graph (bat is sorted); core c owns graphs [32c, 32c+32)
    gpc = G // NCORES
    node_core = bat // gpc
    n_pool = np.bincount(node_core, minlength=NCORES)
    pslot = slot  # gather idx of node n
    p_hi = pslot >= LO
    pk = node_core * 2 + p_hi
    pcnt2 = np.bincount(pk, minlength=NCORES * 2).reshape(NCORES, 2)
    P_LO = int(np.ceil(pcnt2[:, 0].max() / 128))
    P_HI = int(np.ceil(pcnt2[:, 1].max() / 128))
    PCT = P_LO + P_HI
    PS = PCT * 128

    nodes = np.arange(N)
    po = np.lexsort((p_hi, node_core))
    n_o = nodes[po]
    pc_o = node_core[po]
    phi_o = p_hi[po]
    pk_o = pk[po]
    pgrp = np.zeros(NCORES * 2 + 1, np.int64)
    np.cumsum(np.bincount(pk_o, minlength=NCORES * 2), out=pgrp[1:])
    ppos = np.arange(N) - pgrp[pk_o]
    pflat = np.where(phi_o, P_LO * 128, 0) + ppos
    pidx16 = np.zeros((NCORES, PS), np.int16)
    pidx16[pc_o, pflat] = np.where(phi_o, pslot[n_o] - LO, pslot[n_o]).astype(np.int16)
    pdval = np.full((NCORES, 128, PCT), PAD_D, np.float16)
    pq = ppos // 128
    plane = ppos % 128
    pcol = np.where(phi_o, P_LO + pq, pq)
    pdval[pc_o, plane, pcol] = (bat[n_o] - pc_o * gpc).astype(np.float16)

    gcnt = np.bincount(bat, minlength=G).astype(np.float32)

    # per-slot degree (for dis); unused pad slots get deg 1
    deg_slot = np.ones(NCORES * NPC, np.float32)
    deg_slot[slot] = deg.astype(np.float32)
    # reshape per core to [128, TPC] (lane-major per tile)
    degs = np.ones((NCORES, 128, TPC), np.float32)
    for c in range(NCORES):
        d = deg_slot[c * NPC:(c + 1) * NPC]
        degs[c, :, :48] = d[:48 * 128].reshape(48, 128).T
        degs[c, :LAST_CAP, 48] = d[48 * 128:]

    return dict(
        slot=slot, C_LO=C_LO, C_HI=C_HI, CT=CT, ES=ES,
        P_LO=P_LO, P_HI=P_HI, PCT=PCT, PS=PS,
        idx16=idx16, dval=dval, pidx16=pidx16, pdval=pdval,
        gcnt=gcnt, degs=degs, n_pool=n_pool,
    )


def build_nc(C_LO, C_HI, P_LO, P_HI, T_GRP=6, GCH=8, scratch=16384,
             collectives=True, s_dtype="fp16", reps=1):
    """Build the SPMD device program (same NEFF for all 8 cores)."""
    import concourse.bass as bass
    import concourse.mybir as mybir
    import concourse.tile as tile
    from concourse import bacc

    dt = mybir.dt
    Alu = mybir.AluOpType
    Act = mybir.ActivationFunctionType
    CT = C_LO + C_HI
    PCT = P_LO + P_HI
    ES = TPC * CT * 128
    PS = PCT * 128
    caps = _tile_caps()
    sdt = dt.bfloat16 if s_dtype == "bf16" else dt.float16

    nc = bacc.Bacc("TRN2", target_bir_lowering=False, debug=False,
                   num_devices=NCORES if collectives else 1,
                   dynamic_dma_scratch_size=scratch)

    # ---- external inputs ----
    xs = nc.dram_tensor("xs", [NPC, F], dt.float32, kind="ExternalInput")
    degs = nc.dram_tensor("degs", [128, TPC], dt.float32, kind="ExternalInput")
    idx16 = nc.dram_tensor("idx16", [128, ES // 16], dt.int16, kind="ExternalInput")
    dval = nc.dram_tensor("dval", [128, TPC * CT], sdt, kind="ExternalInput")
    pidx16 = nc.dram_tensor("pidx16", [128, PS // 16], dt.int16, kind="ExternalInput")
    pdval = nc.dram_tensor("pdval", [128, PCT], sdt, kind="ExternalInput")
    pcnt = nc.dram_tensor("pcnt", [32, 1], dt.float32, kind="ExternalInput")
    W1 = nc.dram_tensor("W1", [F, F], dt.float32, kind="ExternalInput")
    W2 = nc.dram_tensor("W2", [F, F], dt.float32, kind="ExternalInput")
    Wf1 = nc.dram_tensor("Wf1", [F, F // 2], dt.float32, kind="ExternalInput")
    Wl = nc.dram_tensor("Wl", [F // 2, 2], dt.float32, kind="ExternalInput")
    b1b = nc.dram_tensor("b1b", [128, F], dt.float32, kind="ExternalInput")
    b2b = nc.dram_tensor("b2b", [128, F], dt.float32, kind="ExternalInput")
    bf1c = nc.dram_tensor("bf1c", [F // 2, 1], dt.float32, kind="ExternalInput")
    blc = nc.dram_tensor("blc", [2, 1], dt.float32, kind="ExternalInput")
    iotaT = nc.dram_tensor("iotaT", [128, CT, 128], sdt, kind="ExternalInput")
    piotaT = nc.dram_tensor("piotaT", [128, PCT, 32], sdt, kind="ExternalInput")
    identh = nc.dram_tensor("identh", [128, 128], dt.float16, kind="ExternalInput")
    identf = nc.dram_tensor("identf", [128, 128], dt.float32, kind="ExternalInput")

    out = nc.dram_tensor("out", [G // NCORES, 2], dt.float32, kind="ExternalOutput")

    n_groups = (TPC + T_GRP - 1) // T_GRP

    with tile.TileContext(nc) as tc:
        with (
            tc.tile_pool(name="const", bufs=1) as cp,
            tc.tile_pool(name="meta", bufs=1) as mp,
            tc.tile_pool(name="mlo", bufs=2) as pmlo,
            tc.tile_pool(name="mhi", bufs=2) as pmhi,
            tc.tile_pool(name="sS", bufs=3) as pS,
            tc.tile_pool(name="work", bufs=3) as wp,
            tc.tile_pool(name="ps", bufs=3, space="PSUM") as pp,
            tc.tile_pool(name="pst", bufs=4, space="PSUM") as ppt,
            tc.tile_pool(name="dram", bufs=1, space="DRAM") as dp,
        ):
            # ---- constants ----
            def load_const(pool, src, shape, dtype):
                tl = pool.tile(shape, dtype, tag=src.name)
                nc.sync.dma_start(tl[:], src[:])
                return tl

            iota_sb = load_const(cp, iotaT, [128, CT, 128], sdt)
            piota_sb = load_const(cp, piotaT, [128, PCT, 32], sdt)
            identh_sb = load_const(cp, identh, [128, 128], dt.float16)
            identf_sb = load_const(cp, identf, [128, 128], dt.float32)
            b1b_sb = load_const(cp, b1b, [128, F], dt.float32)
            b2b_sb = load_const(cp, b2b, [128, F], dt.float32)
            bf1_sb = load_const(cp, bf1c, [F // 2, 1], dt.float32)
            bl_sb = load_const(cp, blc, [2, 1], dt.float32)
            idx_sb = load_const(mp, idx16, [128, ES // 16], dt.int16)
            dval_sb = load_const(mp, dval, [128, TPC * CT], sdt)
            pidx_sb = load_const(mp, pidx16, [128, PS // 16], dt.int16)
            pdval_sb = load_const(mp, pdval, [128, PCT], sdt)
            pcnt_sb = load_const(mp, pcnt, [32, 1], dt.float32)

            # weights -> fp16
            def load_w16(src, shape):
                f32t = wp.tile(shape, dt.float32, tag="wtmp")
                nc.sync.dma_start(f32t[:], src[:])
                h = cp.tile(shape, dt.float16, tag=src.name + "h")
                nc.vector.tensor_copy(h[:], f32t[:])
                return h

            W1h = load_w16(W1, [F, F])
            W2h = load_w16(W2, [F, F])
            Wf1h = load_w16(Wf1, [F, F // 2])
            Wlh = load_w16(Wl, [F // 2, 2])

            # dis = (deg>0) * 1/sqrt(max(deg,1))
            deg_sb = load_const(mp, degs, [128, TPC], dt.float32)
            m_sb = wp.tile([128, TPC], dt.float32, tag="dtmp")
            nc.vector.tensor_scalar(m_sb[:], deg_sb[:], 1.0, None, Alu.max)
            r_sb = wp.tile([128, TPC], dt.float32, tag="dtmp2")
            nc.vector.reciprocal(r_sb[:], m_sb[:])
            s_sb = wp.tile([128, TPC], dt.float32, tag="dtmp3")
            nc.scalar.activation(s_sb[:], r_sb[:], Act.Sqrt)
            mask_sb = wp.tile([128, TPC], dt.float32, tag="dtmp4")
            nc.vector.tensor_scalar(mask_sb[:], deg_sb[:], 0.0, None, Alu.is_gt)
            dis_sb = mp.tile([128, TPC], dt.float32, tag="dis")
            nc.vector.tensor_mul(dis_sb[:], s_sb[:], mask_sb[:])

            # ---- DRAM tables ----
            tables = [dp.tile([N, F], dt.float16, tag=f"table{i}", name=f"table{i}")
                      for i in range(4)]
            shards = [dp.tile([NPC, F], dt.float16, tag=f"shard{i}", name=f"shard{i}")
                      for i in range(4)]

            def all_gather(shard, table):
                if collectives:
                    nc.gpsimd.collective_compute(
                        "AllGather", mybir.AluOpType.bypass,
                        ins=[shard.opt()], outs=[table.opt()],
                        replica_groups=[list(range(NCORES))],
                    )
                else:
                    # timing-only stand-in: copy shard into slot-0 region
                    nc.sync.dma_start(table[:NPC, :], shard[:])

            # ---- prologue: T0 = (dis * x) @ W1 ----
            for t in range(TPC):
                cap = caps[t]
                xt = wp.tile([128, F], dt.float32, tag="xt")
                nc.sync.dma_start(xt[:cap, :], xs[t * 128:t * 128 + cap, :])
                xp = wp.tile([128, F], dt.float16, tag="xp")
                nc.vector.tensor_scalar(
                    xp[:cap, :], xt[:cap, :], dis_sb[:cap, t:t + 1], None, Alu.mult)
                xpT_ps = ppt.tile([128, 128], dt.float16, tag="tp")
                nc.tensor.transpose(xpT_ps[:, :cap], xp[:cap, :], identh_sb[:cap, :cap])
                xpT = wp.tile([128, 128], dt.float16, tag="xpT")
                nc.scalar.copy(xpT[:, :cap], xpT_ps[:, :cap])
                T_ps = ppt.tile([128, F], dt.float32, tag="tp")
                nc.tensor.matmul(T_ps[:cap, :], lhsT=xpT[:, :cap], rhs=W1h[:],
                                 start=True, stop=True)
                Tt = wp.tile([128, F], dt.float16, tag="Tt")
                nc.scalar.copy(Tt[:cap, :], T_ps[:cap, :])
                nc.sync.dma_start(shards[0][t * 128:t * 128 + cap, :], Tt[:cap, :])
            all_gather(shards[0], tables[0])

            # ---- conv layers ----
            for l in [ll for _ in range(reps) for ll in range(3)]:
                table = tables[l]
                bias_sb = b1b_sb if l == 0 else b2b_sb
                for g in range(n_groups):
                    t0 = g * T_GRP
                    nt = min(T_GRP, TPC - t0)
                    nlo = nt * C_LO * 128
                    nhi = nt * C_HI * 128
                    mlo = pmlo.tile([128, T_GRP * C_LO, F], dt.float16, tag="mlo")
                    mhi = pmhi.tile([128, T_GRP * C_HI, F], dt.float16, tag="mhi")
                    lo_col0 = (t0 * C_LO * 128) // 16
                    hi_col0 = (TPC * C_LO * 128 + t0 * C_HI * 128) // 16
                    for c0 in range(0, nt * C_LO, GCH):
                        nch = min(GCH, nt * C_LO - c0)
                        nidx = nch * 128
                        nc.gpsimd.dma_gather(
                            mlo[:, c0:c0 + nch, :], table[:LO, :],
                            idx_sb[:, lo_col0 + c0 * 8:lo_col0 + c0 * 8 + nidx // 16],
                            nidx, nidx, F, elem_step=F)
                    for c0 in range(0, nt * C_HI, GCH):
                        nch = min(GCH, nt * C_HI - c0)
                        nidx = nch * 128
                        nc.gpsimd.dma_gather(
                            mhi[:, c0:c0 + nch, :], table[LO:, :],
                            idx_sb[:, hi_col0 + c0 * 8:hi_col0 + c0 * 8 + nidx // 16],
                            nidx, nidx, F, elem_step=F)
                    for ti in range(nt):
                        t = t0 + ti
                        cap = caps[t]
                        S = pS.tile([128, CT, 128], sdt, tag="S")
                        nc.vector.tensor_tensor(
                            S[:, :C_LO, :], iota_sb[:, :C_LO, :],
                            dval_sb[:, t * CT:t * CT + C_LO, None].to_broadcast(
                                [128, C_LO, 128]),
                            op=Alu.is_equal)
                        nc.vector.tensor_tensor(
                            S[:, C_LO:, :], iota_sb[:, C_LO:, :],
                            dval_sb[:, t * CT + C_LO:(t + 1) * CT, None].to_broadcast(
                                [128, C_HI, 128]),
                            op=Alu.is_equal)
                        acc = pp.tile([128, F], dt.float32, tag="acc")
                        for q in range(C_LO):
                            nc.tensor.matmul(
                                acc[:], lhsT=S[:, q, :], rhs=mlo[:, ti * C_LO + q, :],
                                start=(q == 0), stop=False)
                        for qh in range(C_HI):
                            nc.tensor.matmul(
                                acc[:], lhsT=S[:, C_LO + qh, :],
                                rhs=mhi[:, ti * C_HI + qh, :],
                                start=False, stop=(qh == C_HI - 1))
                        # epilogue: y = acc*dis + b ; x' = relu(y)*dis
                        if l < 2:
                            y = wp.tile([128, F], dt.float32, tag="y")
                            nc.vector.scalar_tensor_tensor(
                                y[:], acc[:], dis_sb[:, t:t + 1], bias_sb[:],
                                op0=Alu.mult, op1=Alu.add)
                            xp = wp.tile([128, F], dt.float16, tag="xp")
                            nc.vector.tensor_scalar(
                                xp[:], y[:], 0.0, dis_sb[:, t:t + 1],
                                op0=Alu.max, op1=Alu.mult)
                            xpT_ps = ppt.tile([128, 128], dt.float16, tag="tp")
                            nc.tensor.transpose(
                                xpT_ps[:, :cap], xp[:cap, :], identh_sb[:cap, :cap])
                            xpT = wp.tile([128, 128], dt.float16, tag="xpT")
                            nc.scalar.copy(xpT[:, :cap], xpT_ps[:, :cap])
                            T_ps = ppt.tile([128, F], dt.float32, tag="tp")
                            nc.tensor.matmul(T_ps[:cap, :], lhsT=xpT[:, :cap],
                                             rhs=W2h[:], start=True, stop=True)
                            Tt = wp.tile([128, F], dt.float16, tag="Tt")
                            nc.scalar.copy(Tt[:cap, :], T_ps[:cap, :])
                            nc.sync.dma_start(
                                shards[l + 1][t * 128:t * 128 + cap, :], Tt[:cap, :])
                        else:
                            # y3 (no relu) in fp16 -> pool table
                            y16 = wp.tile([128, F], dt.float16, tag="y16")
                            nc.vector.scalar_tensor_tensor(
                                y16[:cap, :], acc[:cap, :], dis_sb[:cap, t:t + 1],
                                bias_sb[:cap, :], op0=Alu.mult, op1=Alu.add)
                            nc.sync.dma_start(
                                shards[3][t * 128:t * 128 + cap, :], y16[:cap, :])
                all_gather(shards[l + 1], tables[l + 1])

            # ---- pooling ----
            ptab = tables[3]
            pmsg_lo = pmlo.tile([128, P_LO, F], dt.float16, tag="pmlo")
            pmsg_hi = pmhi.tile([128, P_HI, F], dt.float16, tag="pmhi")
            for c0 in range(0, P_LO, GCH):
                nch = min(GCH, P_LO - c0)
                nc.gpsimd.dma_gather(
                    pmsg_lo[:, c0:c0 + nch, :], ptab[:LO, :],
                    pidx_sb[:, c0 * 8:c0 * 8 + nch * 8],
                    nch * 128, nch * 128, F, elem_step=F)
            for c0 in range(0, P_HI, GCH):
                nch = min(GCH, P_HI - c0)
                nc.gpsimd.dma_gather(
                    pmsg_hi[:, c0:c0 + nch, :], ptab[LO:, :],
                    pidx_sb[:, P_LO * 8 + c0 * 8:P_LO * 8 + (c0 + nch) * 8],
                    nch * 128, nch * 128, F, elem_step=F)
            Sp = pS.tile([128, PCT, 32], sdt, tag="Sp")
            nc.vector.tensor_tensor(
                Sp[:, :P_LO, :], piota_sb[:, :P_LO, :],
                pdval_sb[:, :P_LO, None].to_broadcast([128, P_LO, 32]),
                op=Alu.is_equal)
            nc.vector.tensor_tensor(
                Sp[:, P_LO:, :], piota_sb[:, P_LO:, :],
                pdval_sb[:, P_LO:, None].to_broadcast([128, P_HI, 32]),
                op=Alu.is_equal)
            pacc = pp.tile([32, F], dt.float32, tag="acc")
            for q in range(PCT):
                rhs = pmsg_lo[:, q, :] if q < P_LO else pmsg_hi[:, q - P_LO, :]
                nc.tensor.matmul(pacc[:], lhsT=Sp[:, q, :], rhs=rhs,
                                 start=(q == 0), stop=(q == PCT - 1))
            # mean: g = pacc / max(cnt,1)
            cm = wp.tile([32, 1], dt.float32, tag="cm")
            nc.vector.tensor_scalar(cm[:], pcnt_sb[:], 1.0, None, Alu.max)
            cinv = wp.tile([32, 1], dt.float32, tag="cinv")
            nc.vector.reciprocal(cinv[:], cm[:])
            g_sb = wp.tile([32, F], dt.float32, tag="g")
            nc.vector.tensor_scalar(g_sb[:], pacc[:], cinv[:, 0:1], None, Alu.mult)
            # MLP (feature-major): z1T = relu(Wf1^T gT + bf1); z2T = Wl^T z1T + bl
            gT_ps = ppt.tile([128, 32], dt.float32, tag="tp")
            nc.tensor.transpose(gT_ps[:, :32], g_sb[:, :], identf_sb[:32, :32])
            gT_h = wp.tile([128, 32], dt.float16, tag="gTh")
            nc.scalar.copy(gT_h[:], gT_ps[:])
            z1_ps = ppt.tile([F // 2, 32], dt.float32, tag="tp")
            nc.tensor.matmul(z1_ps[:], lhsT=Wf1h[:], rhs=gT_h[:], start=True, stop=True)
            z1_h = wp.tile([F // 2, 32], dt.float16, tag="z1h")
            nc.scalar.activation(z1_h[:], z1_ps[:], Act.Relu, bias=bf1_sb[:, 0:1])
            z2_ps = ppt.tile([2, 32], dt.float32, tag="tp")
            nc.tensor.matmul(z2_ps[:], lhsT=Wlh[:], rhs=z1_h[:], start=True, stop=True)
            z2 = wp.tile([2, 32], dt.float32, tag="z2")
            nc.scalar.activation(z2[:], z2_ps[:], Act.Identity, bias=bl_sb[:, 0:1])
            zT_ps = ppt.tile([32, 2], dt.float32, tag="tp")
            nc.tensor.transpose(zT_ps[:, :2], z2[:, :], identf_sb[:2, :2])
            z_sb = wp.tile([32, 2], dt.float32, tag="zsb")
            nc.scalar.copy(z_sb[:], zT_ps[:])
            nmx = wp.tile([32, 1], dt.float32, tag="nmx")
            nc.vector.tensor_reduce(nmx[:], z_sb[:], axis=mybir.AxisListType.X,
                                    op=Alu.max, negate=True)
            e_sb = wp.tile([32, 2], dt.float32, tag="esb")
            nc.scalar.activation(e_sb[:], z_sb[:], Act.Exp, bias=nmx[:, 0:1])
            sm = wp.tile([32, 1], dt.float32, tag="sm")
            nc.vector.reduce_sum(sm[:], e_sb[:], axis=mybir.AxisListType.X)
            si = wp.tile([32, 1], dt.float32, tag="si")
            nc.vector.reciprocal(si[:], sm[:])
            o_sb = wp.tile([32, 2], dt.float32, tag="o")
            nc.vector.tensor_scalar(o_sb[:], e_sb[:], si[:, 0:1], None, Alu.mult)
            nc.sync.dma_start(out[:], o_sb[:])

    nc.compile()
    return nc


def _host_inputs(inputs, meta, s_dtype="fp16"):
    """Build per-core input maps."""
    import ml_dtypes
    sdt_np = ml_dtypes.bfloat16 if s_dtype == "bf16" else np.float16
    x = np.asarray(inputs["x"], np.float32)
    slot = meta["slot"]
    CT, PCT = meta["CT"], meta["PCT"]
    x_slot = np.empty_like(x)
    x_slot[slot] = x

    iotaT = np.broadcast_to(
        np.arange(128, dtype=sdt_np)[None, None, :], (128, CT, 128)).copy()
    piotaT = np.broadcast_to(
        np.arange(32, dtype=sdt_np)[None, None, :], (128, PCT, 32)).copy()
    identh = np.eye(128, dtype=np.float16)
    identf = np.eye(128, dtype=np.float32)
    b1b = np.broadcast_to(np.asarray(inputs["b1"], np.float32)[None, :], (128, F)).copy()
    b2b = np.broadcast_to(np.asarray(inputs["b2"], np.float32)[None, :], (128, F)).copy()
    bf1c = np.asarray(inputs["bf1"], np.float32).reshape(F // 2, 1)
    blc = np.asarray(inputs["bl"], np.float32).reshape(2, 1)

    shared = dict(
        W1=np.asarray(inputs["W1"], np.float32),
        W2=np.asarray(inputs["W2"], np.float32),
        Wf1=np.asarray(inputs["Wf1"], np.float32),
        Wl=np.asarray(inputs["Wl"], np.float32),
        b1b=b1b, b2b=b2b, bf1c=bf1c, blc=blc,
        iotaT=iotaT, piotaT=piotaT, identh=identh, identf=identf,
    )
    in_maps = []
    gpc = G // NCORES
    for c in range(NCORES):
        m = dict(shared)
        m["xs"] = np.ascontiguousarray(x_slot[c * NPC:(c + 1) * NPC])
        m["degs"] = meta["degs"][c]
        m["idx16"] = _pack_idx16(meta["idx16"][c])
        m["dval"] = meta["dval"][c].astype(sdt_np)
        m["pidx16"] = _pack_idx16(meta["pidx16"][c])
        m["pdval"] = meta["pdval"][c].astype(sdt_np)
        m["pcnt"] = meta["gcnt"][c * gpc:(c + 1) * gpc].reshape(gpc, 1)
        in_maps.append(m)
    return in_maps


def _timeit(nc, in_maps, iters=12):
    """Build the sharded PJRT executable once, run `iters` times, return
    (min, median) wall seconds per execution (incl. dispatch overhead)."""
    import time
    import jax
    import numpy as np
    from jax.sharding import Mesh, PartitionSpec, NamedSharding
    from jax.experimental.shard_map import shard_map
    from concourse import bass2jax
    from concourse import mybir

    bass2jax.install_neuronx_cc_hook()
    partition_name = nc.partition_id_tensor.name if nc.partition_id_tensor else None
    in_names, out_names, out_avals, zero_outs = [], [], [], []
    for alloc in nc.m.functions[0].allocations:
        if not isinstance(alloc, mybir.MemoryLocationSet):
            continue
        name = alloc.memorylocations[0].name
        if alloc.kind == "ExternalInput":
            if name != partition_name:
                in_names.append(name)
        elif alloc.kind == "ExternalOutput":
            shape = tuple(alloc.tensor_shape)
            dtype = mybir.dt.np(alloc.dtype)
            out_names.append(name)
            out_avals.append(jax.core.ShapedArray(shape, dtype))
            zero_outs.append(np.zeros(shape, dtype))
    n_params = len(in_names)
    n_outs = len(out_avals)
    all_in = list(in_names) + list(out_names)
    if partition_name is not None:
        all_in.append(partition_name)
    donate = tuple(range(n_params, n_params + n_outs))

    def _body(*args):
        operands = list(args)
        if partition_name is not None:
            operands.append(bass2jax.partition_id_tensor())
        outs = bass2jax._bass_exec_p.bind(
            *operands, out_avals=tuple(out_avals), in_names=tuple(all_in),
            out_names=tuple(out_names), lowering_input_output_aliases=(),
            sim_require_finite=True, sim_require_nnan=True, nc=nc)
        return tuple(outs)

    devices = jax.devices()[:NCORES]
    mesh = Mesh(np.asarray(devices), ("core",))
    in_specs = (PartitionSpec("core"),) * (n_params + n_outs)
    out_specs = (PartitionSpec("core"),) * len(out_names)
    sharded = jax.jit(
        shard_map(_body, mesh=mesh, in_specs=in_specs, out_specs=out_specs,
                  check_rep=False),
        donate_argnums=donate, keep_unused=True)
    per_core = [[np.asarray(m[name]) for name in in_names] for m in in_maps]
    concat_in = [np.concatenate([per_core[c][i] for c in range(NCORES)], axis=0)
                 for i in range(n_params)]
    sh = NamedSharding(mesh, PartitionSpec("core"))
    dev_in = [jax.device_put(a, sh) for a in concat_in]
    times = []
    out = None
    for i in range(iters):
        zs = [jax.device_put(np.zeros((NCORES * z.shape[0], *z.shape[1:]), z.dtype),
                             sh) for z in zero_outs]
        for z in zs:
            z.block_until_ready()
        t0 = time.perf_counter()
        out = sharded(*dev_in, *zs)
        for o in out:
            o.block_until_ready()
        times.append(time.perf_counter() - t0)
    times.sort()
    res = {name: np.asarray(out[i]).reshape(NCORES, *out_avals[i].shape)
           for i, name in enumerate(out_names)}
    return times[0], times[len(times) // 2], res


def _run(inputs, trace=False):
    from concourse.bass_utils import run_bass_kernel_spmd
    meta = preprocess(np.asarray(inputs["edge_index"]), np.asarray(inputs["batch"]))
    nc = build_nc(meta["C_LO"], meta["C_HI"], meta["P_LO"], meta["P_HI"])
    in_maps = _host_inputs(inputs, meta)
    res = run_bass_kernel_spmd(nc, in_maps, core_ids=list(range(NCORES)),
                               trace=trace)
    outp = np.concatenate([res.results[c]["out"] for c in range(NCORES)], axis=0)
    return outp.astype(np.float32), res, meta


def kernel(**inputs):
    outp, _, _ = _run(inputs, trace=False)
    return outp



# revision 10
# speedup vs baseline: 1.2640x; 1.2640x over previous
"""GCN (3-layer, PyG-style) on 8 Trainium2 NeuronCores.

Strategy (v2): dst-shard nodes across cores (degree-balanced permutation).
Layers 1-2: each core gathers fp16 message rows (dma_gather) for edges
targeting its node range, segment-sums them via one-hot matmuls on the PE
(PSUM accumulation), applies deg-normalization/bias/relu, computes the next
layer's gather table shard (x' @ W), and AllGathers the table.

Layer 3 + mean-pool are algebraically collapsed: the 3rd conv has no relu and
mean-pool is linear, so per-graph sums satisfy
    sum_{i in g} h3[i] = (sum_s C'[s,g] * xp2[s]) @ W2 + n_g * b2,
with xp2[s] = dis_s * relu(h2)[s] (already produced by layer 2's epilogue)
and C'[s,g] = sum_{e: src=s, dst in g} dis[dst_e] host-precomputed. Each core
contracts its xp2 shard against its C' shard (PE matmuls), the [256,128]
partial is AllReduce-summed, and the tiny MLP + softmax run per-core on its
32 graphs; host concatenates per-core [32,2] outputs.
"""
import numpy as np

# problem constants (hardcoded per spec)
N = 50000
E = 800000
F = 128
G = 256
NCORES = 8
NPC = N // NCORES            # 6250 nodes per core
TPC = 49                     # node tiles per core: 48 x 128 + 1 x 106
LAST_CAP = NPC - 48 * 128    # 106
LO = 32768                   # int16 gather index boundary
PAD_D = 255.0                # d_local pad value (never matches iota)
MAX_IDXS = 12288             # per-gather num_idxs cap (Q7 scratch)


def _tile_caps():
    return [128] * 48 + [LAST_CAP]


def _assign_slots(deg):
    """Greedy degree-balanced assignment of nodes to (core, tile, lane) slots.
    Returns slot_of_node [N] (global permuted id)."""
    import heapq
    caps = _tile_caps()
    ntiles = NCORES * TPC
    cap_arr = np.array([caps[t % TPC] for t in range(ntiles)])
    order = np.argsort(-deg, kind="stable")
    heap = [(0, tg) for tg in range(ntiles)]
    heapq.heapify(heap)
    fill = np.zeros(ntiles, np.int64)
    slot = np.empty(N, np.int64)
    for n in order:
        while True:
            s, tg = heapq.heappop(heap)
            if fill[tg] < cap_arr[tg]:
                break
        c, t = tg // TPC, tg % TPC
        slot[n] = c * NPC + t * 128 + fill[tg]
        fill[tg] += 1
        if fill[tg] < cap_arr[tg]:
            heapq.heappush(heap, (s + deg[n], tg))
    assert (fill == cap_arr).all()
    return slot


def _pack_idx16(flat):
    """flat int16 [n] (n % 16 == 0) -> [128, n/16] wrapped+replicated layout."""
    a = flat.reshape(-1, 16).T.astype(np.int16)  # [16, n/16]
    return np.tile(a, (8, 1))


def preprocess(edge_index, batch):
    """All host-side index preprocessing. Returns a dict of per-core and
    shared metadata plus the static schedule constants."""
    src = np.asarray(edge_index[0], dtype=np.int64)
    dst = np.asarray(edge_index[1], dtype=np.int64)
    bat = np.asarray(batch, dtype=np.int64)

    deg = np.bincount(dst, minlength=N).astype(np.int64)
    slot = _assign_slots(deg)

    ps = slot[src]                      # permuted src id
    pd = slot[dst]
    core = pd // NPC
    loc = pd % NPC
    t = np.minimum(loc // 128, 48)
    d_local = loc - t * 128
    is_hi = ps >= LO

    # per (core, tile, half) counts -> static chunk counts
    key = (core * TPC + t) * 2 + is_hi
    cnt = np.bincount(key, minlength=NCORES * TPC * 2).reshape(NCORES, TPC, 2)
    C_LO = int(np.ceil(cnt[:, :, 0].max() / 128))
    C_HI = int(np.ceil(cnt[:, :, 1].max() / 128))
    CT = C_LO + C_HI
    ES = TPC * CT * 128   # edge slots per core

    # order edges by (core, tile, half); stable so deterministic
    eo = np.lexsort((is_hi, t, core))
    ps_o, d_o, core_o, t_o, hi_o = ps[eo], d_local[eo], core[eo], t[eo], is_hi[eo]
    # position within its (core,tile,half) group
    k_o = key[eo]
    grp_start = np.zeros(NCORES * TPC * 2 + 1, np.int64)
    np.cumsum(np.bincount(k_o, minlength=NCORES * TPC * 2), out=grp_start[1:])
    pos = np.arange(E) - grp_start[k_o]

    # flat slot position inside the per-core idx stream
    base_lo = t_o * (C_LO * 128)
    base_hi = TPC * C_LO * 128 + t_o * (C_HI * 128)
    flat = np.where(hi_o, base_hi, base_lo) + pos

    idx16 = np.zeros((NCORES, ES), np.int16)
    idxval = np.where(hi_o, ps_o - LO, ps_o).astype(np.int16)
    idx16[core_o, flat] = idxval
    dval = np.full((NCORES, 128, TPC * CT), PAD_D, np.float16)
    # chunk column for (tile, half, chunk-within-half)
    q = pos // 128
    lane = pos % 128
    col = t_o * CT + np.where(hi_o, C_LO + q, q)
    dval[core_o, lane, col] = d_o.astype(np.float16)

    gcnt = np.bincount(bat, minlength=G).astype(np.float32)

    # per-node dis = (deg>0) / sqrt(max(deg,1))
    dis_n = np.where(deg > 0, 1.0 / np.sqrt(np.maximum(deg, 1.0)), 0.0).astype(
        np.float32)

    # per-slot degree (for dis); unused pad slots get deg 1
    deg_slot = np.ones(NCORES * NPC, np.float32)
    deg_slot[slot] = deg.astype(np.float32)
    # reshape per core to [128, TPC] (lane-major per tile)
    degs = np.ones((NCORES, 128, TPC), np.float32)
    for c in range(NCORES):
        d = deg_slot[c * NPC:(c + 1) * NPC]
        degs[c, :, :48] = d[:48 * 128].reshape(48, 128).T
        degs[c, :LAST_CAP, 48] = d[48 * 128:]

    # ---- layer-3 + pool collapse: C'[slot[s], g] = sum_{e: src=s, dst in g}
    # dis[dst_e] (see module docstring) ----
    key2 = slot[src] * G + bat[dst]
    Cs = np.bincount(key2, weights=dis_n[dst].astype(np.float64),
                     minlength=N * G).reshape(N, G).astype(np.float32)
    Cg = np.zeros((NCORES, TPC * 128, G), np.float16)
    caps = _tile_caps()
    for c in range(NCORES):
        for t_ in range(TPC):
            cap = caps[t_]
            Cg[c, t_ * 128:t_ * 128 + cap] = (
                Cs[c * NPC + t_ * 128: c * NPC + t_ * 128 + cap])

    return dict(
        slot=slot, C_LO=C_LO, C_HI=C_HI, CT=CT, ES=ES,
        idx16=idx16, dval=dval, gcnt=gcnt, degs=degs, Cg=Cg,
    )


def build_nc(C_LO, C_HI, T_GRP=6, scratch=16384, collectives=True,
             s_dtype="fp16"):
    """Build the SPMD device program (same NEFF for all 8 cores)."""
    import concourse.bass as bass
    import concourse.mybir as mybir
    import concourse.tile as tile
    from concourse import bacc

    dt = mybir.dt
    Alu = mybir.AluOpType
    Act = mybir.ActivationFunctionType
    CT = C_LO + C_HI
    ES = TPC * CT * 128
    caps = _tile_caps()
    sdt = dt.bfloat16 if s_dtype == "bf16" else dt.float16
    assert T_GRP * max(C_LO, C_HI) * 128 <= MAX_IDXS

    nc = bacc.Bacc("TRN2", target_bir_lowering=False, debug=False,
                   num_devices=NCORES if collectives else 1,
                   dynamic_dma_scratch_size=scratch)

    # ---- external inputs ----
    xs = nc.dram_tensor("xs", [NPC, F], dt.float32, kind="ExternalInput")
    degs = nc.dram_tensor("degs", [128, TPC], dt.float32, kind="ExternalInput")
    idx16 = nc.dram_tensor("idx16", [128, ES // 16], dt.int16, kind="ExternalInput")
    dval = nc.dram_tensor("dval", [128, TPC * CT], sdt, kind="ExternalInput")
    Cgt = nc.dram_tensor("Cgt", [TPC * 128, G], dt.float16, kind="ExternalInput")
    selT = nc.dram_tensor("selT", [128, 64], dt.float16, kind="ExternalInput")
    b2pool = nc.dram_tensor("b2pool", [G // NCORES, F], dt.float32,
                            kind="ExternalInput")
    pcnt = nc.dram_tensor("pcnt", [G // NCORES, 1], dt.float32,
                          kind="ExternalInput")
    W1 = nc.dram_tensor("W1", [F, F], dt.float32, kind="ExternalInput")
    W2 = nc.dram_tensor("W2", [F, F], dt.float32, kind="ExternalInput")
    Wf1 = nc.dram_tensor("Wf1", [F, F // 2], dt.float32, kind="ExternalInput")
    Wl = nc.dram_tensor("Wl", [F // 2, 2], dt.float32, kind="ExternalInput")
    b1b = nc.dram_tensor("b1b", [128, F], dt.float32, kind="ExternalInput")
    b2b = nc.dram_tensor("b2b", [128, F], dt.float32, kind="ExternalInput")
    bf1c = nc.dram_tensor("bf1c", [F // 2, 1], dt.float32, kind="ExternalInput")
    blc = nc.dram_tensor("blc", [2, 1], dt.float32, kind="ExternalInput")
    iotaT = nc.dram_tensor("iotaT", [128, CT, 128], sdt, kind="ExternalInput")
    identh = nc.dram_tensor("identh", [128, 128], dt.float16, kind="ExternalInput")
    identf = nc.dram_tensor("identf", [128, 128], dt.float32, kind="ExternalInput")

    out = nc.dram_tensor("out", [G // NCORES, 2], dt.float32, kind="ExternalOutput")

    n_groups = (TPC + T_GRP - 1) // T_GRP

    with tile.TileContext(nc) as tc:
        with (
            tc.tile_pool(name="const", bufs=1) as cp,
            tc.tile_pool(name="meta", bufs=1) as mp,
            tc.tile_pool(name="mlo", bufs=2) as pmlo,
            tc.tile_pool(name="mhi", bufs=2) as pmhi,
            tc.tile_pool(name="sS", bufs=3) as pS,
            tc.tile_pool(name="cgp", bufs=3) as pcg,
            tc.tile_pool(name="work", bufs=3) as wp,
            tc.tile_pool(name="ps", bufs=2, space="PSUM") as pp,
            tc.tile_pool(name="pst", bufs=3, space="PSUM") as ppt,
            tc.tile_pool(name="psP", bufs=1, space="PSUM") as ppP,
            tc.tile_pool(name="dram", bufs=1, space="DRAM") as dp,
        ):
            # ---- constants ----
            def load_const(pool, src, shape, dtype):
                tl = pool.tile(shape, dtype, tag=src.name)
                nc.sync.dma_start(tl[:], src[:])
                return tl

            iota_sb = load_const(cp, iotaT, [128, CT, 128], sdt)
            identh_sb = load_const(cp, identh, [128, 128], dt.float16)
            identf_sb = load_const(cp, identf, [128, 128], dt.float32)
            b1b_sb = load_const(cp, b1b, [128, F], dt.float32)
            b2b_sb = load_const(cp, b2b, [128, F], dt.float32)
            bf1_sb = load_const(cp, bf1c, [F // 2, 1], dt.float32)
            bl_sb = load_const(cp, blc, [2, 1], dt.float32)
            sel_sb = load_const(cp, selT, [128, 64], dt.float16)
            b2p_sb = load_const(cp, b2pool, [G // NCORES, F], dt.float32)
            idx_sb = load_const(mp, idx16, [128, ES // 16], dt.int16)
            dval_sb = load_const(mp, dval, [128, TPC * CT], sdt)
            pcnt_sb = load_const(mp, pcnt, [G // NCORES, 1], dt.float32)
            zero_sb = cp.tile([128, F], dt.float32, tag="zeros")
            nc.vector.memset(zero_sb[:], 0.0)

            # weights -> fp16
            def load_w16(src, shape):
                f32t = wp.tile(shape, dt.float32, tag="wtmp")
                nc.sync.dma_start(f32t[:], src[:])
                h = cp.tile(shape, dt.float16, tag=src.name + "h")
                nc.vector.tensor_copy(h[:], f32t[:])
                return h

            W1h = load_w16(W1, [F, F])
            W2h = load_w16(W2, [F, F])
            Wf1h = load_w16(Wf1, [F, F // 2])
            Wlh = load_w16(Wl, [F // 2, 2])

            # dis = (deg>0) * 1/sqrt(max(deg,1))
            deg_sb = load_const(mp, degs, [128, TPC], dt.float32)
            m_sb = wp.tile([128, TPC], dt.float32, tag="dtmp")
            nc.vector.tensor_scalar(m_sb[:], deg_sb[:], 1.0, None, Alu.max)
            r_sb = wp.tile([128, TPC], dt.float32, tag="dtmp2")
            nc.vector.reciprocal(r_sb[:], m_sb[:])
            s_sb = wp.tile([128, TPC], dt.float32, tag="dtmp3")
            nc.scalar.activation(s_sb[:], r_sb[:], Act.Sqrt)
            mask_sb = wp.tile([128, TPC], dt.float32, tag="dtmp4")
            nc.vector.tensor_scalar(mask_sb[:], deg_sb[:], 0.0, None, Alu.is_gt)
            dis_sb = mp.tile([128, TPC], dt.float32, tag="dis")
            nc.vector.tensor_mul(dis_sb[:], s_sb[:], mask_sb[:])

            # ---- DRAM tables ----
            tables = [dp.tile([N, F], dt.float16, tag=f"table{i}", name=f"table{i}")
                      for i in range(2)]
            shards = [dp.tile([NPC, F], dt.float16, tag=f"shard{i}", name=f"shard{i}")
                      for i in range(2)]
            Pd = dp.tile([128, 2 * F], dt.float32, tag="Pd", name="Pd")
            Pr = dp.tile([128, 2 * F], dt.float32, tag="Pr", name="Pr")

            def all_gather(shard, table):
                if collectives:
                    nc.gpsimd.collective_compute(
                        "AllGather", mybir.AluOpType.bypass,
                        ins=[shard.opt()], outs=[table.opt()],
                        replica_groups=[list(range(NCORES))],
                    )
                else:
                    nc.sync.dma_start(table[:NPC, :], shard[:])

            # ---- prologue: T0 = (dis * x) @ W1 ----
            for t in range(TPC):
                cap = caps[t]
                xt = wp.tile([128, F], dt.float32, tag="xt")
                nc.sync.dma_start(xt[:cap, :], xs[t * 128:t * 128 + cap, :])
                xp = wp.tile([128, F], dt.float16, tag="xp")
                nc.vector.tensor_scalar(
                    xp[:cap, :], xt[:cap, :], dis_sb[:cap, t:t + 1], None, Alu.mult)
                xpT_ps = ppt.tile([128, 128], dt.float16, tag="tp")
                nc.tensor.transpose(xpT_ps[:, :cap], xp[:cap, :], identh_sb[:cap, :cap])
                xpT = wp.tile([128, 128], dt.float16, tag="xpT")
                nc.scalar.copy(xpT[:, :cap], xpT_ps[:, :cap])
                T_ps = ppt.tile([128, F], dt.float32, tag="tp")
                nc.tensor.matmul(T_ps[:cap, :], lhsT=xpT[:, :cap], rhs=W1h[:],
                                 start=True, stop=True)
                Tt = wp.tile([128, F], dt.float16, tag="Tt")
                nc.scalar.copy(Tt[:cap, :], T_ps[:cap, :])
                nc.sync.dma_start(shards[0][t * 128:t * 128 + cap, :], Tt[:cap, :])
            all_gather(shards[0], tables[0])

            # layer-2 xp tiles are kept resident; the pooled partial
            # P = C'^T @ xp2 runs as one clean accumulation pass afterwards
            xp_keep = mp.tile([128, TPC, F], dt.float16, tag="xpkeep")

            # ---- conv layers 1-2 ----
            for l in range(2):
                table = tables[l]
                bias_sb = b1b_sb if l == 0 else b2b_sb
                for g in range(n_groups):
                    t0 = g * T_GRP
                    nt = min(T_GRP, TPC - t0)
                    mlo = pmlo.tile([128, T_GRP * C_LO, F], dt.float16, tag="mlo")
                    mhi = pmhi.tile([128, T_GRP * C_HI, F], dt.float16, tag="mhi")
                    lo_col0 = (t0 * C_LO * 128) // 16
                    hi_col0 = (TPC * C_LO * 128 + t0 * C_HI * 128) // 16
                    GCH = 8
                    for c0 in range(0, nt * C_LO, GCH):
                        nch = min(GCH, nt * C_LO - c0)
                        nidx = nch * 128
                        nc.gpsimd.dma_gather(
                            mlo[:, c0:c0 + nch, :], table[:LO, :],
                            idx_sb[:, lo_col0 + c0 * 8:lo_col0 + c0 * 8 + nidx // 16],
                            nidx, nidx, F, elem_step=F)
                    for c0 in range(0, nt * C_HI, GCH):
                        nch = min(GCH, nt * C_HI - c0)
                        nidx = nch * 128
                        nc.gpsimd.dma_gather(
                            mhi[:, c0:c0 + nch, :], table[LO:, :],
                            idx_sb[:, hi_col0 + c0 * 8:hi_col0 + c0 * 8 + nidx // 16],
                            nidx, nidx, F, elem_step=F)
                    for ti in range(nt):
                        t = t0 + ti
                        cap = caps[t]
                        S = pS.tile([128, CT, 128], sdt, tag="S")
                        nc.vector.tensor_tensor(
                            S[:, :C_LO, :], iota_sb[:, :C_LO, :],
                            dval_sb[:, t * CT:t * CT + C_LO, None].to_broadcast(
                                [128, C_LO, 128]),
                            op=Alu.is_equal)
                        nc.vector.tensor_tensor(
                            S[:, C_LO:, :], iota_sb[:, C_LO:, :],
                            dval_sb[:, t * CT + C_LO:(t + 1) * CT, None].to_broadcast(
                                [128, C_HI, 128]),
                            op=Alu.is_equal)
                        acc = pp.tile([128, F], dt.float32, tag="acc")
                        for q in range(C_LO):
                            nc.tensor.matmul(
                                acc[:], lhsT=S[:, q, :], rhs=mlo[:, ti * C_LO + q, :],
                                start=(q == 0), stop=False)
                        for qh in range(C_HI):
                            nc.tensor.matmul(
                                acc[:], lhsT=S[:, C_LO + qh, :],
                                rhs=mhi[:, ti * C_HI + qh, :],
                                start=False, stop=(qh == C_HI - 1))
                        # epilogue: y = acc*dis + b ; xp = relu(y*dis) = relu(y)*dis
                        y = wp.tile([128, F], dt.float32, tag="y")
                        nc.vector.scalar_tensor_tensor(
                            y[:], acc[:], dis_sb[:, t:t + 1], bias_sb[:],
                            op0=Alu.mult, op1=Alu.add)
                        if l == 0:
                            xp = wp.tile([128, F], dt.float16, tag="xp2")
                            nc.vector.scalar_tensor_tensor(
                                xp[:], y[:], dis_sb[:, t:t + 1], zero_sb[:],
                                op0=Alu.mult, op1=Alu.max)
                            xpT_ps = ppt.tile([128, 128], dt.float16, tag="tp")
                            nc.tensor.transpose(
                                xpT_ps[:, :cap], xp[:cap, :], identh_sb[:cap, :cap])
                            xpT = wp.tile([128, 128], dt.float16, tag="xpT")
                            nc.scalar.copy(xpT[:, :cap], xpT_ps[:, :cap])
                            T_ps = ppt.tile([128, F], dt.float32, tag="tp")
                            nc.tensor.matmul(T_ps[:cap, :], lhsT=xpT[:, :cap],
                                             rhs=W2h[:], start=True, stop=True)
                            Tt = wp.tile([128, F], dt.float16, tag="Tt")
                            nc.scalar.copy(Tt[:cap, :], T_ps[:cap, :])
                            nc.sync.dma_start(
                                shards[1][t * 128:t * 128 + cap, :], Tt[:cap, :])
                        else:
                            nc.vector.scalar_tensor_tensor(
                                xp_keep[:, t, :], y[:], dis_sb[:, t:t + 1],
                                zero_sb[:], op0=Alu.mult, op1=Alu.max)
                if l == 0:
                    all_gather(shards[1], tables[1])

            # ---- pooled partial: P = C'^T @ xp2 (clean accumulation pass) ----
            P0 = ppP.tile([128, F], dt.float32, tag="P0")  # graphs 0..127
            P1 = ppP.tile([128, F], dt.float32, tag="P1")  # graphs 128..255
            for t in range(TPC):
                cg_t = pcg.tile([128, G], dt.float16, tag="cg")
                nc.sync.dma_start(cg_t[:], Cgt[t * 128:(t + 1) * 128, :])
                nc.tensor.matmul(P0[:], lhsT=cg_t[:, 0:128], rhs=xp_keep[:, t, :],
                                 start=(t == 0), stop=(t == TPC - 1))
                nc.tensor.matmul(P1[:], lhsT=cg_t[:, 128:256], rhs=xp_keep[:, t, :],
                                 start=(t == 0), stop=(t == TPC - 1))

            # ---- pooled head: AllReduce P, extract my 32 graphs, @W2, MLP ----
            P_sb = wp.tile([128, 2 * F], dt.float32, tag="Psb")
            nc.scalar.copy(P_sb[:, 0:F], P0[:])
            nc.scalar.copy(P_sb[:, F:2 * F], P1[:])
            nc.sync.dma_start(Pd[:], P_sb[:])
            if collectives:
                nc.gpsimd.collective_compute(
                    "AllReduce", mybir.AluOpType.add,
                    ins=[Pd.opt()], outs=[Pr.opt()],
                    replica_groups=[list(range(NCORES))],
                )
            else:
                nc.sync.dma_start(Pr[:], Pd[:])
            Pr_f = wp.tile([128, 2 * F], dt.float32, tag="Prf")
            nc.sync.dma_start(Pr_f[:], Pr[:])
            Pr_h = wp.tile([128, 2 * F], dt.float16, tag="Prh")
            nc.vector.tensor_copy(Pr_h[:], Pr_f[:])
            # my 32 graphs: P_my[j,:] = sum_p sel0[p,j]*P0[p,:] + sel1[p,j]*P1[p,:]
            pmy_ps = ppt.tile([32, F], dt.float32, tag="tp")
            nc.tensor.matmul(pmy_ps[:], lhsT=sel_sb[:, 0:32], rhs=Pr_h[:, 0:F],
                             start=True, stop=False)
            nc.tensor.matmul(pmy_ps[:], lhsT=sel_sb[:, 32:64], rhs=Pr_h[:, F:2 * F],
                             start=False, stop=True)
            pmy_sb = wp.tile([32, F], dt.float16, tag="pmy")
            nc.scalar.copy(pmy_sb[:], pmy_ps[:])
            pmyT_ps = ppt.tile([128, 32], dt.float16, tag="tp")
            nc.tensor.transpose(pmyT_ps[:, :32], pmy_sb[:, :], identh_sb[:32, :32])
            pmyT = wp.tile([128, 32], dt.float16, tag="pmyT")
            nc.scalar.copy(pmyT[:], pmyT_ps[:])
            G_ps = ppt.tile([32, F], dt.float32, tag="tp")
            nc.tensor.matmul(G_ps[:], lhsT=pmyT[:, :32], rhs=W2h[:],
                             start=True, stop=True)
            # g = G*cinv + b2pool ; then MLP
            cm = wp.tile([32, 1], dt.float32, tag="cm")
            nc.vector.tensor_scalar(cm[:], pcnt_sb[:], 1.0, None, Alu.max)
            cinv = wp.tile([32, 1], dt.float32, tag="cinv")
            nc.vector.reciprocal(cinv[:], cm[:])
            g_sb = wp.tile([32, F], dt.float32, tag="g")
            nc.vector.scalar_tensor_tensor(
                g_sb[:], G_ps[:], cinv[:, 0:1], b2p_sb[:],
                op0=Alu.mult, op1=Alu.add)
            # MLP (feature-major): z1T = relu(Wf1^T gT + bf1); z2T = Wl^T z1T + bl
            gT_ps = ppt.tile([128, 32], dt.float32, tag="tp")
            nc.tensor.transpose(gT_ps[:, :32], g_sb[:, :], identf_sb[:32, :32])
            gT_h = wp.tile([128, 32], dt.float16, tag="gTh")
            nc.scalar.copy(gT_h[:], gT_ps[:])
            z1_ps = ppt.tile([F // 2, 32], dt.float32, tag="tp")
            nc.tensor.matmul(z1_ps[:], lhsT=Wf1h[:], rhs=gT_h[:], start=True, stop=True)
            z1_h = wp.tile([F // 2, 32], dt.float16, tag="z1h")
            nc.scalar.activation(z1_h[:], z1_ps[:], Act.Relu, bias=bf1_sb[:, 0:1])
            z2_ps = ppt.tile([2, 32], dt.float32, tag="tp")
            nc.tensor.matmul(z2_ps[:], lhsT=Wlh[:], rhs=z1_h[:], start=True, stop=True)
            z2 = wp.tile([2, 32], dt.float32, tag="z2")
            nc.scalar.activation(z2[:], z2_ps[:], Act.Identity, bias=bl_sb[:, 0:1])
            zT_ps = ppt.tile([32, 2], dt.float32, tag="tp")
            nc.tensor.transpose(zT_ps[:, :2], z2[:, :], identf_sb[:2, :2])
            z_sb = wp.tile([32, 2], dt.float32, tag="zsb")
            nc.scalar.copy(z_sb[:], zT_ps[:])
            nmx = wp.tile([32, 1], dt.float32, tag="nmx")
            nc.vector.tensor_reduce(nmx[:], z_sb[:], axis=mybir.AxisListType.X,
                                    op=Alu.max, negate=True)
            e_sb = wp.tile([32, 2], dt.float32, tag="esb")
            nc.scalar.activation(e_sb[:], z_sb[:], Act.Exp, bias=nmx[:, 0:1])
            sm = wp.tile([32, 1], dt.float32, tag="sm")
            nc.vector.reduce_sum(sm[:], e_sb[:], axis=mybir.AxisListType.X)
            si = wp.tile([32, 1], dt.float32, tag="si")
            nc.vector.reciprocal(si[:], sm[:])
            o_sb = wp.tile([32, 2], dt.float32, tag="o")
            nc.vector.tensor_scalar(o_sb[:], e_sb[:], si[:, 0:1], None, Alu.mult)
            nc.sync.dma_start(out[:], o_sb[:])

    nc.compile()
    return nc


def _host_inputs(inputs, meta, s_dtype="fp16"):
    """Build per-core input maps."""
    import ml_dtypes
    sdt_np = ml_dtypes.bfloat16 if s_dtype == "bf16" else np.float16
    x = np.asarray(inputs["x"], np.float32)
    slot = meta["slot"]
    CT = meta["CT"]
    x_slot = np.empty_like(x)
    x_slot[slot] = x

    iotaT = np.broadcast_to(
        np.arange(128, dtype=sdt_np)[None, None, :], (128, CT, 128)).copy()
    identh = np.eye(128, dtype=np.float16)
    identf = np.eye(128, dtype=np.float32)
    b1b = np.broadcast_to(np.asarray(inputs["b1"], np.float32)[None, :], (128, F)).copy()
    b2b = np.broadcast_to(np.asarray(inputs["b2"], np.float32)[None, :], (128, F)).copy()
    bf1c = np.asarray(inputs["bf1"], np.float32).reshape(F // 2, 1)
    blc = np.asarray(inputs["bl"], np.float32).reshape(2, 1)
    b2 = np.asarray(inputs["b2"], np.float32)
    gcnt = meta["gcnt"]

    shared = dict(
        W1=np.asarray(inputs["W1"], np.float32),
        W2=np.asarray(inputs["W2"], np.float32),
        Wf1=np.asarray(inputs["Wf1"], np.float32),
        Wl=np.asarray(inputs["Wl"], np.float32),
        b1b=b1b, b2b=b2b, bf1c=bf1c, blc=blc,
        iotaT=iotaT, identh=identh, identf=identf,
    )
    in_maps = []
    gpc = G // NCORES
    for c in range(NCORES):
        m = dict(shared)
        m["xs"] = np.ascontiguousarray(x_slot[c * NPC:(c + 1) * NPC])
        m["degs"] = meta["degs"][c]
        m["idx16"] = _pack_idx16(meta["idx16"][c])
        m["dval"] = meta["dval"][c].astype(sdt_np)
        m["Cgt"] = np.ascontiguousarray(meta["Cg"][c])
        m["pcnt"] = gcnt[c * gpc:(c + 1) * gpc].reshape(gpc, 1)
        # selection one-hots for this core's 32 graphs out of the two
        # 128-graph halves of the AllReduced P
        selT = np.zeros((128, 64), np.float16)
        for j in range(gpc):
            gg = c * gpc + j
            if gg < 128:
                selT[gg, j] = 1.0
            else:
                selT[gg - 128, 32 + j] = 1.0
        m["selT"] = selT
        bfac = (gcnt[c * gpc:(c + 1) * gpc] > 0).astype(np.float32)
        m["b2pool"] = np.ascontiguousarray(bfac[:, None] * b2[None, :])
        in_maps.append(m)
    return in_maps


def _run(inputs, trace=False):
    from concourse.bass_utils import run_bass_kernel_spmd
    meta = preprocess(np.asarray(inputs["edge_index"]), np.asarray(inputs["batch"]))
    nc = build_nc(meta["C_LO"], meta["C_HI"])
    in_maps = _host_inputs(inputs, meta)
    res = run_bass_kernel_spmd(nc, in_maps, core_ids=list(range(NCORES)),
                               trace=trace)
    outp = np.concatenate([res.results[c]["out"] for c in range(NCORES)], axis=0)
    return outp.astype(np.float32), res, meta


def kernel(**inputs):
    outp, _, _ = _run(inputs, trace=False)
    return outp


# revision 11
# speedup vs baseline: 1.6138x; 1.2767x over previous
"""GCN (3-layer, PyG-style) on 8 Trainium2 NeuronCores.

Strategy (v2): dst-shard nodes across cores (degree-balanced permutation).
Layers 1-2: each core gathers fp16 message rows (dma_gather) for edges
targeting its node range, segment-sums them via one-hot matmuls on the PE
(PSUM accumulation), applies deg-normalization/bias/relu, computes the next
layer's gather table shard (x' @ W), and AllGathers the table.

Layer 3 + mean-pool are algebraically collapsed: the 3rd conv has no relu and
mean-pool is linear, so per-graph sums satisfy
    sum_{i in g} h3[i] = (sum_s C'[s,g] * xp2[s]) @ W2 + n_g * b2,
with xp2[s] = dis_s * relu(h2)[s] (already produced by layer 2's epilogue)
and C'[s,g] = sum_{e: src=s, dst in g} dis[dst_e] host-precomputed. Each core
contracts its xp2 shard against its C' shard (PE matmuls), the [256,128]
partial is AllReduce-summed, and the tiny MLP + softmax run per-core on its
32 graphs; host concatenates per-core [32,2] outputs.
"""
import numpy as np

# problem constants (hardcoded per spec)
N = 50000
E = 800000
F = 128
G = 256
NCORES = 8
NPC = N // NCORES            # 6250 nodes per core
TPC = 49                     # node tiles per core: 48 x 128 + 1 x 106
LAST_CAP = NPC - 48 * 128    # 106
LO = 32768                   # int16 gather index boundary
PAD_D = 255.0                # d_local pad value (never matches iota)
MAX_IDXS = 12288             # per-gather num_idxs cap (Q7 scratch)


def _tile_caps():
    return [128] * 48 + [LAST_CAP]


def _assign_slots(deg):
    """Greedy degree-balanced assignment of nodes to (core, tile, lane) slots.
    Returns slot_of_node [N] (global permuted id)."""
    import heapq
    caps = _tile_caps()
    ntiles = NCORES * TPC
    cap_arr = np.array([caps[t % TPC] for t in range(ntiles)])
    order = np.argsort(-deg, kind="stable")
    heap = [(0, tg) for tg in range(ntiles)]
    heapq.heapify(heap)
    fill = np.zeros(ntiles, np.int64)
    slot = np.empty(N, np.int64)
    for n in order:
        while True:
            s, tg = heapq.heappop(heap)
            if fill[tg] < cap_arr[tg]:
                break
        c, t = tg // TPC, tg % TPC
        slot[n] = c * NPC + t * 128 + fill[tg]
        fill[tg] += 1
        if fill[tg] < cap_arr[tg]:
            heapq.heappush(heap, (s + deg[n], tg))
    assert (fill == cap_arr).all()
    return slot


def _pack_idx16(flat):
    """flat int16 [n] (n % 16 == 0) -> [128, n/16] wrapped+replicated layout."""
    a = flat.reshape(-1, 16).T.astype(np.int16)  # [16, n/16]
    return np.tile(a, (8, 1))


def preprocess(edge_index, batch):
    """All host-side index preprocessing. Returns a dict of per-core and
    shared metadata plus the static schedule constants."""
    src = np.asarray(edge_index[0], dtype=np.int64)
    dst = np.asarray(edge_index[1], dtype=np.int64)
    bat = np.asarray(batch, dtype=np.int64)

    deg = np.bincount(dst, minlength=N).astype(np.int64)
    slot = _assign_slots(deg)

    ps = slot[src]                      # permuted src id
    pd = slot[dst]
    core = pd // NPC
    loc = pd % NPC
    t = np.minimum(loc // 128, 48)
    d_local = loc - t * 128
    is_hi = ps >= LO

    # per (core, tile, half) counts -> static chunk counts
    key = (core * TPC + t) * 2 + is_hi
    cnt = np.bincount(key, minlength=NCORES * TPC * 2).reshape(NCORES, TPC, 2)
    C_LO = int(np.ceil(cnt[:, :, 0].max() / 128))
    C_HI = int(np.ceil(cnt[:, :, 1].max() / 128))
    CT = C_LO + C_HI
    ES = TPC * CT * 128   # edge slots per core

    # order edges by (core, tile, half); stable so deterministic
    eo = np.lexsort((is_hi, t, core))
    ps_o, d_o, core_o, t_o, hi_o = ps[eo], d_local[eo], core[eo], t[eo], is_hi[eo]
    # position within its (core,tile,half) group
    k_o = key[eo]
    grp_start = np.zeros(NCORES * TPC * 2 + 1, np.int64)
    np.cumsum(np.bincount(k_o, minlength=NCORES * TPC * 2), out=grp_start[1:])
    pos = np.arange(E) - grp_start[k_o]

    # flat slot position inside the per-core idx stream
    base_lo = t_o * (C_LO * 128)
    base_hi = TPC * C_LO * 128 + t_o * (C_HI * 128)
    flat = np.where(hi_o, base_hi, base_lo) + pos

    idx16 = np.zeros((NCORES, ES), np.int16)
    idxval = np.where(hi_o, ps_o - LO, ps_o).astype(np.int16)
    idx16[core_o, flat] = idxval
    dval = np.full((NCORES, 128, TPC * CT), PAD_D, np.float16)
    # chunk column for (tile, half, chunk-within-half)
    q = pos // 128
    lane = pos % 128
    col = t_o * CT + np.where(hi_o, C_LO + q, q)
    dval[core_o, lane, col] = d_o.astype(np.float16)

    gcnt = np.bincount(bat, minlength=G).astype(np.float32)

    # per-node dis = (deg>0) / sqrt(max(deg,1))
    dis_n = np.where(deg > 0, 1.0 / np.sqrt(np.maximum(deg, 1.0)), 0.0).astype(
        np.float32)

    # per-slot degree (for dis); unused pad slots get deg 1
    deg_slot = np.ones(NCORES * NPC, np.float32)
    deg_slot[slot] = deg.astype(np.float32)
    # reshape per core to [128, TPC] (lane-major per tile)
    degs = np.ones((NCORES, 128, TPC), np.float32)
    for c in range(NCORES):
        d = deg_slot[c * NPC:(c + 1) * NPC]
        degs[c, :, :48] = d[:48 * 128].reshape(48, 128).T
        degs[c, :LAST_CAP, 48] = d[48 * 128:]

    # ---- layer-3 + pool collapse: C'[slot[s], g] = sum_{e: src=s, dst in g}
    # dis[dst_e] (see module docstring) ----
    key2 = slot[src] * G + bat[dst]
    Cs = np.bincount(key2, weights=dis_n[dst].astype(np.float64),
                     minlength=N * G).reshape(N, G).astype(np.float32)
    Cg = np.zeros((NCORES, TPC * 128, G), np.float16)
    caps = _tile_caps()
    for c in range(NCORES):
        for t_ in range(TPC):
            cap = caps[t_]
            Cg[c, t_ * 128:t_ * 128 + cap] = (
                Cs[c * NPC + t_ * 128: c * NPC + t_ * 128 + cap])

    return dict(
        slot=slot, C_LO=C_LO, C_HI=C_HI, CT=CT, ES=ES,
        idx16=idx16, dval=dval, gcnt=gcnt, degs=degs, Cg=Cg,
    )


def build_nc(C_LO, C_HI, T_GRP=6, scratch=65536, collectives=True,
             s_dtype="fp16"):
    """Build the SPMD device program (same NEFF for all 8 cores)."""
    import concourse.bass as bass
    import concourse.mybir as mybir
    import concourse.tile as tile
    from concourse import bacc

    dt = mybir.dt
    Alu = mybir.AluOpType
    Act = mybir.ActivationFunctionType
    CT = C_LO + C_HI
    ES = TPC * CT * 128
    caps = _tile_caps()
    sdt = dt.bfloat16 if s_dtype == "bf16" else dt.float16
    assert T_GRP * max(C_LO, C_HI) * 128 <= MAX_IDXS

    nc = bacc.Bacc("TRN2", target_bir_lowering=False, debug=False,
                   num_devices=NCORES if collectives else 1,
                   dynamic_dma_scratch_size=scratch)

    # ---- external inputs ----
    xs = nc.dram_tensor("xs", [NPC, F], dt.float32, kind="ExternalInput")
    degs = nc.dram_tensor("degs", [128, TPC], dt.float32, kind="ExternalInput")
    idx16 = nc.dram_tensor("idx16", [128, ES // 16], dt.int16, kind="ExternalInput")
    dval = nc.dram_tensor("dval", [128, TPC * CT], sdt, kind="ExternalInput")
    Cgt = nc.dram_tensor("Cgt", [TPC * 128, G], dt.float16, kind="ExternalInput")
    selT = nc.dram_tensor("selT", [128, 64], dt.float16, kind="ExternalInput")
    b2pool = nc.dram_tensor("b2pool", [G // NCORES, F], dt.float32,
                            kind="ExternalInput")
    pcnt = nc.dram_tensor("pcnt", [G // NCORES, 1], dt.float32,
                          kind="ExternalInput")
    W1 = nc.dram_tensor("W1", [F, F], dt.float32, kind="ExternalInput")
    W2 = nc.dram_tensor("W2", [F, F], dt.float32, kind="ExternalInput")
    Wf1 = nc.dram_tensor("Wf1", [F, F // 2], dt.float32, kind="ExternalInput")
    Wl = nc.dram_tensor("Wl", [F // 2, 2], dt.float32, kind="ExternalInput")
    b1b = nc.dram_tensor("b1b", [128, F], dt.float32, kind="ExternalInput")
    b2b = nc.dram_tensor("b2b", [128, F], dt.float32, kind="ExternalInput")
    bf1c = nc.dram_tensor("bf1c", [F // 2, 1], dt.float32, kind="ExternalInput")
    blc = nc.dram_tensor("blc", [2, 1], dt.float32, kind="ExternalInput")
    iotaT = nc.dram_tensor("iotaT", [128, CT, 128], sdt, kind="ExternalInput")
    identh = nc.dram_tensor("identh", [128, 128], dt.float16, kind="ExternalInput")
    identf = nc.dram_tensor("identf", [128, 128], dt.float32, kind="ExternalInput")

    out = nc.dram_tensor("out", [G // NCORES, 2], dt.float32, kind="ExternalOutput")

    n_groups = (TPC + T_GRP - 1) // T_GRP

    with tile.TileContext(nc) as tc:
        with (
            tc.tile_pool(name="const", bufs=1) as cp,
            tc.tile_pool(name="meta", bufs=1) as mp,
            tc.tile_pool(name="mlo", bufs=2) as pmlo,
            tc.tile_pool(name="mhi", bufs=2) as pmhi,
            tc.tile_pool(name="sS", bufs=3) as pS,
            tc.tile_pool(name="cgp", bufs=3) as pcg,
            tc.tile_pool(name="work", bufs=3) as wp,
            tc.tile_pool(name="ps", bufs=2, space="PSUM") as pp,
            tc.tile_pool(name="pst", bufs=3, space="PSUM") as ppt,
            tc.tile_pool(name="psP", bufs=1, space="PSUM") as ppP,
            tc.tile_pool(name="dram", bufs=1, space="DRAM") as dp,
        ):
            # ---- constants ----
            def load_const(pool, src, shape, dtype):
                tl = pool.tile(shape, dtype, tag=src.name)
                nc.sync.dma_start(tl[:], src[:])
                return tl

            iota_sb = load_const(cp, iotaT, [128, CT, 128], sdt)
            identh_sb = load_const(cp, identh, [128, 128], dt.float16)
            identf_sb = load_const(cp, identf, [128, 128], dt.float32)
            b1b_sb = load_const(cp, b1b, [128, F], dt.float32)
            b2b_sb = load_const(cp, b2b, [128, F], dt.float32)
            bf1_sb = load_const(cp, bf1c, [F // 2, 1], dt.float32)
            bl_sb = load_const(cp, blc, [2, 1], dt.float32)
            sel_sb = load_const(cp, selT, [128, 64], dt.float16)
            b2p_sb = load_const(cp, b2pool, [G // NCORES, F], dt.float32)
            idx_sb = load_const(mp, idx16, [128, ES // 16], dt.int16)
            dval_sb = load_const(mp, dval, [128, TPC * CT], sdt)
            pcnt_sb = load_const(mp, pcnt, [G // NCORES, 1], dt.float32)
            zero_sb = cp.tile([128, F], dt.float32, tag="zeros")
            nc.vector.memset(zero_sb[:], 0.0)

            # weights -> fp16
            def load_w16(src, shape):
                f32t = wp.tile(shape, dt.float32, tag="wtmp")
                nc.sync.dma_start(f32t[:], src[:])
                h = cp.tile(shape, dt.float16, tag=src.name + "h")
                nc.vector.tensor_copy(h[:], f32t[:])
                return h

            W1h = load_w16(W1, [F, F])
            W2h = load_w16(W2, [F, F])
            Wf1h = load_w16(Wf1, [F, F // 2])
            Wlh = load_w16(Wl, [F // 2, 2])

            # dis = (deg>0) * 1/sqrt(max(deg,1))
            deg_sb = load_const(mp, degs, [128, TPC], dt.float32)
            m_sb = wp.tile([128, TPC], dt.float32, tag="dtmp")
            nc.vector.tensor_scalar(m_sb[:], deg_sb[:], 1.0, None, Alu.max)
            r_sb = wp.tile([128, TPC], dt.float32, tag="dtmp2")
            nc.vector.reciprocal(r_sb[:], m_sb[:])
            s_sb = wp.tile([128, TPC], dt.float32, tag="dtmp3")
            nc.scalar.activation(s_sb[:], r_sb[:], Act.Sqrt)
            mask_sb = wp.tile([128, TPC], dt.float32, tag="dtmp4")
            nc.vector.tensor_scalar(mask_sb[:], deg_sb[:], 0.0, None, Alu.is_gt)
            dis_sb = mp.tile([128, TPC], dt.float32, tag="dis")
            nc.vector.tensor_mul(dis_sb[:], s_sb[:], mask_sb[:])

            # ---- DRAM tables ----
            tables = [dp.tile([N, F], dt.float16, tag=f"table{i}", name=f"table{i}")
                      for i in range(2)]
            shards = [dp.tile([NPC, F], dt.float16, tag=f"shard{i}", name=f"shard{i}")
                      for i in range(2)]
            Pd = dp.tile([128, 2 * F], dt.float32, tag="Pd", name="Pd")
            Pr = dp.tile([128, 2 * F], dt.float32, tag="Pr", name="Pr")

            def all_gather(shard, table):
                if collectives:
                    nc.gpsimd.collective_compute(
                        "AllGather", mybir.AluOpType.bypass,
                        ins=[shard.opt()], outs=[table.opt()],
                        replica_groups=[list(range(NCORES))],
                    )
                else:
                    nc.sync.dma_start(table[:NPC, :], shard[:])

            # ---- prologue: T0 = (dis * x) @ W1 ----
            for t in range(TPC):
                cap = caps[t]
                xt = wp.tile([128, F], dt.float32, tag="xt")
                nc.sync.dma_start(xt[:cap, :], xs[t * 128:t * 128 + cap, :])
                xp = wp.tile([128, F], dt.float16, tag="xp")
                nc.vector.tensor_scalar(
                    xp[:cap, :], xt[:cap, :], dis_sb[:cap, t:t + 1], None, Alu.mult)
                xpT_ps = ppt.tile([128, 128], dt.float16, tag="tp")
                nc.tensor.transpose(xpT_ps[:, :cap], xp[:cap, :], identh_sb[:cap, :cap])
                xpT = wp.tile([128, 128], dt.float16, tag="xpT")
                nc.scalar.copy(xpT[:, :cap], xpT_ps[:, :cap])
                T_ps = ppt.tile([128, F], dt.float32, tag="tp")
                nc.tensor.matmul(T_ps[:cap, :], lhsT=xpT[:, :cap], rhs=W1h[:],
                                 start=True, stop=True)
                Tt = wp.tile([128, F], dt.float16, tag="Tt")
                nc.scalar.copy(Tt[:cap, :], T_ps[:cap, :])
                nc.sync.dma_start(shards[0][t * 128:t * 128 + cap, :], Tt[:cap, :])
            all_gather(shards[0], tables[0])

            # layer-2 xp tiles are kept resident; the pooled partial
            # P = C'^T @ xp2 runs as one clean accumulation pass afterwards
            xp_keep = mp.tile([128, TPC, F], dt.float16, tag="xpkeep")

            # ---- conv layers 1-2 ----
            for l in range(2):
                table = tables[l]
                bias_sb = b1b_sb if l == 0 else b2b_sb
                for g in range(n_groups):
                    t0 = g * T_GRP
                    nt = min(T_GRP, TPC - t0)
                    mlo = pmlo.tile([128, T_GRP * C_LO, F], dt.float16, tag="mlo")
                    mhi = pmhi.tile([128, T_GRP * C_HI, F], dt.float16, tag="mhi")
                    lo_col0 = (t0 * C_LO * 128) // 16
                    hi_col0 = (TPC * C_LO * 128 + t0 * C_HI * 128) // 16
                    GCH = 36  # 4608 idx = 290 descs/engine; needs big ring,
                    # single_packet=False (64-desc packet ceiling)
                    for c0 in range(0, nt * C_LO, GCH):
                        nch = min(GCH, nt * C_LO - c0)
                        nidx = nch * 128
                        nc.gpsimd.dma_gather(
                            mlo[:, c0:c0 + nch, :], table[:LO, :],
                            idx_sb[:, lo_col0 + c0 * 8:lo_col0 + c0 * 8 + nidx // 16],
                            nidx, nidx, F, elem_step=F, single_packet=False)
                    for c0 in range(0, nt * C_HI, GCH):
                        nch = min(GCH, nt * C_HI - c0)
                        nidx = nch * 128
                        nc.gpsimd.dma_gather(
                            mhi[:, c0:c0 + nch, :], table[LO:, :],
                            idx_sb[:, hi_col0 + c0 * 8:hi_col0 + c0 * 8 + nidx // 16],
                            nidx, nidx, F, elem_step=F, single_packet=False)
                    for ti in range(nt):
                        t = t0 + ti
                        cap = caps[t]
                        S = pS.tile([128, CT, 128], sdt, tag="S")
                        nc.vector.tensor_tensor(
                            S[:, :C_LO, :], iota_sb[:, :C_LO, :],
                            dval_sb[:, t * CT:t * CT + C_LO, None].to_broadcast(
                                [128, C_LO, 128]),
                            op=Alu.is_equal)
                        nc.vector.tensor_tensor(
                            S[:, C_LO:, :], iota_sb[:, C_LO:, :],
                            dval_sb[:, t * CT + C_LO:(t + 1) * CT, None].to_broadcast(
                                [128, C_HI, 128]),
                            op=Alu.is_equal)
                        acc = pp.tile([128, F], dt.float32, tag="acc")
                        for q in range(C_LO):
                            nc.tensor.matmul(
                                acc[:], lhsT=S[:, q, :], rhs=mlo[:, ti * C_LO + q, :],
                                start=(q == 0), stop=False)
                        for qh in range(C_HI):
                            nc.tensor.matmul(
                                acc[:], lhsT=S[:, C_LO + qh, :],
                                rhs=mhi[:, ti * C_HI + qh, :],
                                start=False, stop=(qh == C_HI - 1))
                        # epilogue: y = acc*dis + b ; xp = relu(y*dis) = relu(y)*dis
                        y = wp.tile([128, F], dt.float32, tag="y")
                        nc.vector.scalar_tensor_tensor(
                            y[:], acc[:], dis_sb[:, t:t + 1], bias_sb[:],
                            op0=Alu.mult, op1=Alu.add)
                        if l == 0:
                            xp = wp.tile([128, F], dt.float16, tag="xp2")
                            nc.vector.scalar_tensor_tensor(
                                xp[:], y[:], dis_sb[:, t:t + 1], zero_sb[:],
                                op0=Alu.mult, op1=Alu.max)
                            xpT_ps = ppt.tile([128, 128], dt.float16, tag="tp")
                            nc.tensor.transpose(
                                xpT_ps[:, :cap], xp[:cap, :], identh_sb[:cap, :cap])
                            xpT = wp.tile([128, 128], dt.float16, tag="xpT")
                            nc.scalar.copy(xpT[:, :cap], xpT_ps[:, :cap])
                            T_ps = ppt.tile([128, F], dt.float32, tag="tp")
                            nc.tensor.matmul(T_ps[:cap, :], lhsT=xpT[:, :cap],
                                             rhs=W2h[:], start=True, stop=True)
                            Tt = wp.tile([128, F], dt.float16, tag="Tt")
                            nc.scalar.copy(Tt[:cap, :], T_ps[:cap, :])
                            nc.sync.dma_start(
                                shards[1][t * 128:t * 128 + cap, :], Tt[:cap, :])
                        else:
                            nc.vector.scalar_tensor_tensor(
                                xp_keep[:, t, :], y[:], dis_sb[:, t:t + 1],
                                zero_sb[:], op0=Alu.mult, op1=Alu.max)
                if l == 0:
                    all_gather(shards[1], tables[1])

            # ---- pooled partial: P = C'^T @ xp2 (clean accumulation pass) ----
            P0 = ppP.tile([128, F], dt.float32, tag="P0")  # graphs 0..127
            P1 = ppP.tile([128, F], dt.float32, tag="P1")  # graphs 128..255
            for t in range(TPC):
                cg_t = pcg.tile([128, G], dt.float16, tag="cg")
                nc.sync.dma_start(cg_t[:], Cgt[t * 128:(t + 1) * 128, :])
                nc.tensor.matmul(P0[:], lhsT=cg_t[:, 0:128], rhs=xp_keep[:, t, :],
                                 start=(t == 0), stop=(t == TPC - 1))
                nc.tensor.matmul(P1[:], lhsT=cg_t[:, 128:256], rhs=xp_keep[:, t, :],
                                 start=(t == 0), stop=(t == TPC - 1))

            # ---- pooled head: AllReduce P, extract my 32 graphs, @W2, MLP ----
            P_sb = wp.tile([128, 2 * F], dt.float32, tag="Psb")
            nc.scalar.copy(P_sb[:, 0:F], P0[:])
            nc.scalar.copy(P_sb[:, F:2 * F], P1[:])
            nc.sync.dma_start(Pd[:], P_sb[:])
            if collectives:
                nc.gpsimd.collective_compute(
                    "AllReduce", mybir.AluOpType.add,
                    ins=[Pd.opt()], outs=[Pr.opt()],
                    replica_groups=[list(range(NCORES))],
                )
            else:
                nc.sync.dma_start(Pr[:], Pd[:])
            Pr_f = wp.tile([128, 2 * F], dt.float32, tag="Prf")
            nc.sync.dma_start(Pr_f[:], Pr[:])
            Pr_h = wp.tile([128, 2 * F], dt.float16, tag="Prh")
            nc.vector.tensor_copy(Pr_h[:], Pr_f[:])
            # my 32 graphs: P_my[j,:] = sum_p sel0[p,j]*P0[p,:] + sel1[p,j]*P1[p,:]
            pmy_ps = ppt.tile([32, F], dt.float32, tag="tp")
            nc.tensor.matmul(pmy_ps[:], lhsT=sel_sb[:, 0:32], rhs=Pr_h[:, 0:F],
                             start=True, stop=False)
            nc.tensor.matmul(pmy_ps[:], lhsT=sel_sb[:, 32:64], rhs=Pr_h[:, F:2 * F],
                             start=False, stop=True)
            pmy_sb = wp.tile([32, F], dt.float16, tag="pmy")
            nc.scalar.copy(pmy_sb[:], pmy_ps[:])
            pmyT_ps = ppt.tile([128, 32], dt.float16, tag="tp")
            nc.tensor.transpose(pmyT_ps[:, :32], pmy_sb[:, :], identh_sb[:32, :32])
            pmyT = wp.tile([128, 32], dt.float16, tag="pmyT")
            nc.scalar.copy(pmyT[:], pmyT_ps[:])
            G_ps = ppt.tile([32, F], dt.float32, tag="tp")
            nc.tensor.matmul(G_ps[:], lhsT=pmyT[:, :32], rhs=W2h[:],
                             start=True, stop=True)
            # g = G*cinv + b2pool ; then MLP
            cm = wp.tile([32, 1], dt.float32, tag="cm")
            nc.vector.tensor_scalar(cm[:], pcnt_sb[:], 1.0, None, Alu.max)
            cinv = wp.tile([32, 1], dt.float32, tag="cinv")
            nc.vector.reciprocal(cinv[:], cm[:])
            g_sb = wp.tile([32, F], dt.float32, tag="g")
            nc.vector.scalar_tensor_tensor(
                g_sb[:], G_ps[:], cinv[:, 0:1], b2p_sb[:],
                op0=Alu.mult, op1=Alu.add)
            # MLP (feature-major): z1T = relu(Wf1^T gT + bf1); z2T = Wl^T z1T + bl
            gT_ps = ppt.tile([128, 32], dt.float32, tag="tp")
            nc.tensor.transpose(gT_ps[:, :32], g_sb[:, :], identf_sb[:32, :32])
            gT_h = wp.tile([128, 32], dt.float16, tag="gTh")
            nc.scalar.copy(gT_h[:], gT_ps[:])
            z1_ps = ppt.tile([F // 2, 32], dt.float32, tag="tp")
            nc.tensor.matmul(z1_ps[:], lhsT=Wf1h[:], rhs=gT_h[:], start=True, stop=True)
            z1_h = wp.tile([F // 2, 32], dt.float16, tag="z1h")
            nc.scalar.activation(z1_h[:], z1_ps[:], Act.Relu, bias=bf1_sb[:, 0:1])
            z2_ps = ppt.tile([2, 32], dt.float32, tag="tp")
            nc.tensor.matmul(z2_ps[:], lhsT=Wlh[:], rhs=z1_h[:], start=True, stop=True)
            z2 = wp.tile([2, 32], dt.float32, tag="z2")
            nc.scalar.activation(z2[:], z2_ps[:], Act.Identity, bias=bl_sb[:, 0:1])
            zT_ps = ppt.tile([32, 2], dt.float32, tag="tp")
            nc.tensor.transpose(zT_ps[:, :2], z2[:, :], identf_sb[:2, :2])
            z_sb = wp.tile([32, 2], dt.float32, tag="zsb")
            nc.scalar.copy(z_sb[:], zT_ps[:])
            nmx = wp.tile([32, 1], dt.float32, tag="nmx")
            nc.vector.tensor_reduce(nmx[:], z_sb[:], axis=mybir.AxisListType.X,
                                    op=Alu.max, negate=True)
            e_sb = wp.tile([32, 2], dt.float32, tag="esb")
            nc.scalar.activation(e_sb[:], z_sb[:], Act.Exp, bias=nmx[:, 0:1])
            sm = wp.tile([32, 1], dt.float32, tag="sm")
            nc.vector.reduce_sum(sm[:], e_sb[:], axis=mybir.AxisListType.X)
            si = wp.tile([32, 1], dt.float32, tag="si")
            nc.vector.reciprocal(si[:], sm[:])
            o_sb = wp.tile([32, 2], dt.float32, tag="o")
            nc.vector.tensor_scalar(o_sb[:], e_sb[:], si[:, 0:1], None, Alu.mult)
            nc.sync.dma_start(out[:], o_sb[:])

    nc.compile()
    return nc


def _host_inputs(inputs, meta, s_dtype="fp16"):
    """Build per-core input maps."""
    import ml_dtypes
    sdt_np = ml_dtypes.bfloat16 if s_dtype == "bf16" else np.float16
    x = np.asarray(inputs["x"], np.float32)
    slot = meta["slot"]
    CT = meta["CT"]
    x_slot = np.empty_like(x)
    x_slot[slot] = x

    iotaT = np.broadcast_to(
        np.arange(128, dtype=sdt_np)[None, None, :], (128, CT, 128)).copy()
    identh = np.eye(128, dtype=np.float16)
    identf = np.eye(128, dtype=np.float32)
    b1b = np.broadcast_to(np.asarray(inputs["b1"], np.float32)[None, :], (128, F)).copy()
    b2b = np.broadcast_to(np.asarray(inputs["b2"], np.float32)[None, :], (128, F)).copy()
    bf1c = np.asarray(inputs["bf1"], np.float32).reshape(F // 2, 1)
    blc = np.asarray(inputs["bl"], np.float32).reshape(2, 1)
    b2 = np.asarray(inputs["b2"], np.float32)
    gcnt = meta["gcnt"]

    shared = dict(
        W1=np.asarray(inputs["W1"], np.float32),
        W2=np.asarray(inputs["W2"], np.float32),
        Wf1=np.asarray(inputs["Wf1"], np.float32),
        Wl=np.asarray(inputs["Wl"], np.float32),
        b1b=b1b, b2b=b2b, bf1c=bf1c, blc=blc,
        iotaT=iotaT, identh=identh, identf=identf,
    )
    in_maps = []
    gpc = G // NCORES
    for c in range(NCORES):
        m = dict(shared)
        m["xs"] = np.ascontiguousarray(x_slot[c * NPC:(c + 1) * NPC])
        m["degs"] = meta["degs"][c]
        m["idx16"] = _pack_idx16(meta["idx16"][c])
        m["dval"] = meta["dval"][c].astype(sdt_np)
        m["Cgt"] = np.ascontiguousarray(meta["Cg"][c])
        m["pcnt"] = gcnt[c * gpc:(c + 1) * gpc].reshape(gpc, 1)
        # selection one-hots for this core's 32 graphs out of the two
        # 128-graph halves of the AllReduced P
        selT = np.zeros((128, 64), np.float16)
        for j in range(gpc):
            gg = c * gpc + j
            if gg < 128:
                selT[gg, j] = 1.0
            else:
                selT[gg - 128, 32 + j] = 1.0
        m["selT"] = selT
        bfac = (gcnt[c * gpc:(c + 1) * gpc] > 0).astype(np.float32)
        m["b2pool"] = np.ascontiguousarray(bfac[:, None] * b2[None, :])
        in_maps.append(m)
    return in_maps


def _run(inputs, trace=False):
    from concourse.bass_utils import run_bass_kernel_spmd
    meta = preprocess(np.asarray(inputs["edge_index"]), np.asarray(inputs["batch"]))
    nc = build_nc(meta["C_LO"], meta["C_HI"])
    in_maps = _host_inputs(inputs, meta)
    res = run_bass_kernel_spmd(nc, in_maps, core_ids=list(range(NCORES)),
                               trace=trace)
    outp = np.concatenate([res.results[c]["out"] for c in range(NCORES)], axis=0)
    return outp.astype(np.float32), res, meta


def kernel(**inputs):
    outp, _, _ = _run(inputs, trace=False)
    return outp
